# revision 6
# baseline (speedup 1.0000x reference)
"""Trainium2 Bass kernel for DurationCalculator (histogram binning).

Contract: kernel(att_ws, ilens, olens) -> durations (B, T) float32, matching

    diag = att_ws.max(-1).mean(-1).mean(0); head = argmax(diag)
    aw = att_ws[:, head]; amax = argmax(where(t < ilen, aw, -inf), -1)
    durations[b, i] = sum_l (amax[b, l] == i) * (l < olen[b])

Sharding: data-parallel over batch B across 8 NeuronCores (B_local = 2).

Phase A streams each (b, h) slice as one contiguous 1 MB DMA laid out as
partition p = l // 8 (8 KB contiguous per partition line) and max-reduces
over T on VectorE.  Per-head partial sums cross a single AllReduce; each
core then argmaxes the 16 diag scores on-device (vector max/max_index).

Phase B re-reads only the selected head (one register-offset DMA per
sample), masks invalid encoder columns with a min-cap, recovers the
first-index argmax per decoder frame via an equality mask against the row
max weighted by a reversed iota, and bins the indices with a one-hot
compare + fp32 matmul against the row-validity weights in PSUM.
"""

import sys

if "/opt/trn_rl_repo" not in sys.path:
    sys.path.insert(0, "/opt/trn_rl_repo")

import numpy as np

B, H, L, T = 16, 16, 1024, 256
N_CORES = 8
B_LOC = B // N_CORES  # 2 samples per core
P = 128
J = L // P  # 8 decoder frames per partition (l = 8*p + j)

_cache: dict = {}


def _build_program():
    import concourse.bass as bass
    import concourse.bacc as bacc
    import concourse.mybir as mybir
    from concourse.tile import TileContext

    fp32 = mybir.dt.float32
    u32 = mybir.dt.uint32
    Alu = mybir.AluOpType
    Ax = mybir.AxisListType

    nc = bacc.Bacc(num_devices=N_CORES)

    att = nc.dram_tensor("att", [B_LOC, H, L, T], fp32, kind="ExternalInput")
    # columns: [ilen0, ilen1, olen0, olen1], replicated across 128 partitions
    lens_f = nc.dram_tensor("lens_f", [P, 2 * B_LOC], fp32, kind="ExternalInput")
    iota_t = nc.dram_tensor("iota_t", [1, T], fp32, kind="ExternalInput")
    iota_p = nc.dram_tensor("iota_p", [P, 1], fp32, kind="ExternalInput")
    dur = nc.dram_tensor("dur", [B_LOC, T], fp32, kind="ExternalOutput")
    cc_in = nc.dram_tensor("cc_in", [1, H], fp32)
    cc_out = nc.dram_tensor("cc_out", [1, H], fp32, addr_space="Shared")

    att_handle = att

    def bh_slice_ap(offset_expr):
        # (128, 8*T) view of one (b, h) slice: partition p holds rows
        # l = 8p .. 8p+7, 2048 contiguous elements per partition.
        return bass.AP(
            tensor=att_handle, offset=offset_expr, ap=[[J * T, P], [1, J * T]]
        )

    with TileContext(nc) as tc:
        with (
            tc.tile_pool(name="stream", bufs=8) as stream,
            tc.tile_pool(name="consts", bufs=1) as consts,
            tc.tile_pool(name="small", bufs=4) as small,
            tc.tile_pool(name="phb", bufs=1) as phb,
            tc.tile_pool(name="psum", bufs=4, space="PSUM") as psum,
        ):
            # ---------------- constants ----------------
            iota_tf = consts.tile([P, T], fp32)
            it = iota_t[:]
            nc.sync.dma_start(
                out=iota_tf[:],
                in_=bass.AP(tensor=it.tensor, offset=it.offset, ap=[[0, P], [1, T]]),
            )
            iota_pf = consts.tile([P, 1], fp32)
            nc.sync.dma_start(out=iota_pf[:], in_=iota_p[:])
            lens_sb = consts.tile([P, 2 * B_LOC], fp32)
            nc.sync.dma_start(out=lens_sb[:], in_=lens_f[:])
            ones_col = consts.tile([P, 1], fp32)
            nc.vector.memset(ones_col[:], 1.0)
            # revT[p, t] = T - t ; m8p[p] = -8p
            revT = consts.tile([P, T], fp32)
            nc.vector.tensor_scalar(
                out=revT[:], in0=iota_tf[:], scalar1=-1.0, scalar2=float(T),
                op0=Alu.mult, op1=Alu.add,
            )
            m8p = consts.tile([P, 1], fp32)
            nc.vector.tensor_scalar(
                out=m8p[:], in0=iota_pf[:], scalar1=-float(J), scalar2=None,
                op0=Alu.mult,
            )

            # ---------------- phase A: diag scores ----------------
            # maxbuf[p, b*128 + h*8 + j] = max_t att[b, h, 8p + j, t]
            maxbuf = consts.tile([P, B_LOC * H * J], fp32)
            for b in range(B_LOC):
                for h in range(H):
                    data = stream.tile([P, J * T], fp32)
                    nc.sync.dma_start(
                        out=data[:], in_=bh_slice_ap((b * H + h) * (L * T))
                    )
                    col = (b * H + h) * J
                    d3 = bass.AP(
                        tensor=data[:].tensor,
                        offset=data[:].offset,
                        ap=[data[:].ap[0], [T, J], [1, T]],
                    )
                    nc.vector.tensor_reduce(
                        out=maxbuf[:, col : col + J], in_=d3, axis=Ax.X, op=Alu.max
                    )

            # per-head sums over (b, j) then over partitions via ones-matmul
            view = maxbuf[:].rearrange("p (b h j) -> p h b j", b=B_LOC, h=H, j=J)
            hsum = small.tile([P, H], fp32)
            nc.vector.tensor_reduce(out=hsum[:], in_=view, axis=Ax.XY, op=Alu.add)
            diag_ps = psum.tile([1, H], fp32, space="PSUM")
            nc.tensor.matmul(
                out=diag_ps[:], lhsT=ones_col[:], rhs=hsum[:], start=True, stop=True
            )
            diag_sb = small.tile([1, H], fp32)
            nc.vector.tensor_copy(out=diag_sb[:], in_=diag_ps[:])

            # ---------------- AllReduce over the 8 cores ----------------
            nc.sync.dma_start(out=cc_in[:], in_=diag_sb[:])
            nc.gpsimd.collective_compute(
                "AllReduce",
                Alu.add,
                replica_groups=[list(range(N_CORES))],
                ins=[cc_in[:]],
                outs=[cc_out[:]],
            )
            diag_all = small.tile([1, H], fp32)
            nc.sync.dma_start(out=diag_all[:], in_=cc_out[:])

            # head = argmax(diag) on one partition
            dtop = small.tile([1, 8], fp32)
            nc.vector.max(out=dtop[:], in_=diag_all[:])
            didx = small.tile([1, 8], u32)
            nc.vector.max_index(out=didx[:], in_max=dtop[:], in_values=diag_all[:])

            # ---------------- phase B: masked argmax + histogram ----------------
            hreg = nc.sync.alloc_register("head")
            nc.sync.reg_load(hreg, didx[0:1, 0:1])
            head_val = nc.sync.snap(hreg, min_val=0, max_val=H - 1)

            def b3(tile_ap, dims):
                return bass.AP(tensor=tile_ap.tensor, offset=tile_ap.offset, ap=dims)

            for b in range(B_LOC):
                # cap[p, t] = 2.0 if t < ilen[b] else -1.0  (att values are in [0, 1))
                cap = small.tile([P, T], fp32)
                nc.vector.tensor_scalar(
                    out=cap[:], in0=iota_tf[:], scalar1=lens_sb[:, b : b + 1],
                    scalar2=None, op0=Alu.is_lt,
                )
                nc.vector.tensor_scalar(
                    out=cap[:], in0=cap[:], scalar1=3.0, scalar2=-1.0,
                    op0=Alu.mult, op1=Alu.add,
                )
                # row-validity weights w8[p, j] = (8p + j < olen[b])
                olm = small.tile([P, J], fp32)
                nc.vector.tensor_scalar(
                    out=olm[:], in0=iota_tf[:, 0:J],
                    scalar1=lens_sb[:, 2 + b : 3 + b], scalar2=None, op0=Alu.subtract,
                )
                w8 = small.tile([P, J], fp32)
                nc.vector.tensor_scalar(
                    out=w8[:], in0=olm[:], scalar1=m8p[:], scalar2=None, op0=Alu.is_lt
                )

                aw = stream.tile([P, J * T], fp32)
                nc.sync.dma_start(
                    out=aw[:],
                    in_=bh_slice_ap(head_val * (L * T) + b * (H * L * T)),
                )

                awap = aw[:]
                p_ap = awap.ap[0]
                masked = phb.tile([P, J * T], fp32)
                nc.vector.tensor_tensor(
                    out=masked[:],
                    in0=awap,
                    in1=b3(cap[:], [cap[:].ap[0], [0, J], [1, T]]),
                    op=Alu.min,
                )
                m3 = b3(masked[:], [masked[:].ap[0], [T, J], [1, T]])
                rmax = small.tile([P, J], fp32)
                nc.vector.tensor_reduce(out=rmax[:], in_=m3, axis=Ax.X, op=Alu.max)
                # first-index argmax: eqm * (T - t), max over t, idx = T - that
                eqm = phb.tile([P, J * T], fp32)
                nc.vector.tensor_tensor(
                    out=eqm[:],
                    in0=masked[:],
                    in1=b3(rmax[:], [rmax[:].ap[0], [1, J], [0, T]]),
                    op=Alu.is_equal,
                )
                score = phb.tile([P, J * T], fp32)
                nc.vector.tensor_tensor(
                    out=score[:],
                    in0=eqm[:],
                    in1=b3(revT[:], [revT[:].ap[0], [0, J], [1, T]]),
                    op=Alu.mult,
                )
                s3 = b3(score[:], [score[:].ap[0], [T, J], [1, T]])
                sidx = small.tile([P, J], fp32)
                nc.vector.tensor_reduce(out=sidx[:], in_=s3, axis=Ax.X, op=Alu.max)
                idxf8 = small.tile([P, J], fp32)
                nc.vector.tensor_scalar(
                    out=idxf8[:], in0=sidx[:], scalar1=-1.0, scalar2=float(T),
                    op0=Alu.mult, op1=Alu.add,
                )
                # one-hot over T per (p, j), weighted by w8 inside the matmul
                eqh = phb.tile([P, J, T], fp32)
                nc.vector.tensor_tensor(
                    out=eqh[:],
                    in0=b3(iota_tf[:], [iota_tf[:].ap[0], [0, J], [1, T]]),
                    in1=b3(idxf8[:], [idxf8[:].ap[0], [1, J], [0, T]]),
                    op=Alu.is_equal,
                )
                hist_ps = psum.tile([1, T], fp32, space="PSUM")
                for j in range(J):
                    nc.tensor.matmul(
                        out=hist_ps[:],
                        lhsT=w8[:, j : j + 1],
                        rhs=eqh[:, j, :],
                        start=(j == 0),
                        stop=(j == J - 1),
                    )
                out_sb = small.tile([1, T], fp32)
                nc.vector.tensor_copy(out=out_sb[:], in_=hist_ps[:])
                nc.sync.dma_start(out=dur[b : b + 1, :], in_=out_sb[:])

    # Bacc passes split multi-sem waits (HW allows 1 wait per DMA trigger),
    # allocate registers, and fuse nops — required before serializing BIR.
    nc.compile()
    return nc


def get_program():
    if "nc" not in _cache:
        _cache["nc"] = _build_program()
    return _cache["nc"]


def make_in_maps(att_ws, ilens, olens):
    att_ws = np.asarray(att_ws, dtype=np.float32)
    ilens = np.asarray(ilens, dtype=np.int32)
    olens = np.asarray(olens, dtype=np.int32)
    iota_t = np.arange(T, dtype=np.float32).reshape(1, T)
    iota_p = np.arange(P, dtype=np.float32).reshape(P, 1)
    in_maps = []
    for c in range(N_CORES):
        sl = slice(c * B_LOC, (c + 1) * B_LOC)
        lens_row = np.concatenate(
            [ilens[sl].astype(np.float32), olens[sl].astype(np.float32)]
        )
        in_maps.append(
            {
                "att": np.ascontiguousarray(att_ws[sl]),
                "lens_f": np.ascontiguousarray(
                    np.broadcast_to(lens_row, (P, 2 * B_LOC))
                ),
                "iota_t": iota_t,
                "iota_p": iota_p,
            }
        )
    return in_maps


def kernel(att_ws, ilens, olens):
    from concourse.bass_utils import run_bass_kernel_spmd
    import os

    nc = get_program()
    in_maps = make_in_maps(att_ws, ilens, olens)
    trace = bool(int(os.environ.get("BASS_KERNEL_TRACE", "0")))
    res = run_bass_kernel_spmd(nc, in_maps, core_ids=list(range(N_CORES)), trace=trace)
    _cache["last_results"] = res
    out = np.concatenate([res.results[c]["dur"] for c in range(N_CORES)], axis=0)
    return out.astype(np.float32)
